# revision 5
# baseline (speedup 1.0000x reference)
"""Trainium2 Bass kernel for nn_AttentionV2 (dense transformer attention block).

Reference computation (per batch element b):
    q  = Wq @ x_b  + qb          # [128, 4096]  (1x1 conv over channels)
    k  = Wk @ aux_b + kb         # [128, 4096]
    v  = Wv @ aux_b + vb         # [128, 4096]
    ktq[i, j] = sum_c k[c, i] * q[c, j]          # [4096, 4096]
    atten = softmax(ktq, axis=j)
    y[c, j] = sum_i v[c, i] * atten[i, j]        # [128, 4096]
    z = Wz @ y + zb + x_b        # [256, 4096]

Sharding: batch B=8 across the 8 cores (data parallel, weights replicated).
Each core runs the whole attention for its batch element; no collectives.

Per-core design notes:
  * All matmuls contract over the partition dim; k/q land as [c=128, hw] so
    ktq tiles need no transposes.  v is produced directly transposed
    (vT[i, c]) by using aux as the stationary operand, so the attention
    matmul y = vT.T @ exp needs no transpose either.
  * Softmax is computed unnormalized (no max subtraction: |ktq| <~ 30 so
    exp stays finite in fp32); the 1/rowsum factor is folded into vT
    (per-partition scalar multiply), which makes normalization free.
  * ScalarE computes exp straight out of PSUM with accum_out producing the
    row sums, so the only DVE work is the y accumulation flushes.
  * PSUM budget (8 banks): 2 x [128, 1536] ktq chunks (6 banks) feed exp;
    2 x [128, 512] (2 banks) rotate for the grouped y accumulation and the
    final z conv.
  * i-tiles are processed in groups of G=3; the y matmuls of group g are
    interleaved (in emission order) with the ktq/exp work of group g+1 so
    the tensor engine keeps ScalarE fed at the group boundary.
"""

import sys

if "/opt/trn_rl_repo" not in sys.path:
    sys.path.insert(0, "/opt/trn_rl_repo")

import numpy as np

import concourse.bass as bass
import concourse.bacc as bacc
import concourse.mybir as mybir
import concourse.tile as tile
from concourse.masks import make_identity

DT = mybir.dt.float32
P = 128          # partitions
C = 256          # input channels
CH = 128         # conv output channels (C//2)
HW = 4096        # 64*64 spatial
NJB = HW // 512  # 8 column blocks of 512
NIT = HW // P    # 32 i-tiles
G = 3            # i-tiles per y-accumulation group
# exp is computed in chunks straight out of PSUM; chunk layout per i-tile:
EXP_CHUNKS = ((0, 1536), (1536, 1536), (3072, 1024))
KT_SLOT = 1536   # psum slot width for ktq chunks (3 banks)

EXP_BUFS = 5
VTS_BUFS = 6

Exp = mybir.ActivationFunctionType.Exp
Identity = mybir.ActivationFunctionType.Identity
AX = mybir.AxisListType.X


def _groups():
    return [list(range(s, min(s + G, NIT))) for s in range(0, NIT, G)]


def build_module() -> bass.Bass:
    # Bacc (not plain Bass): its compile() pipeline moves extra matmul waits
    # onto LDWEIGHTS and splits >1-wait instructions (TRN2 ISA allows one
    # sync wait per instruction) -- walrus rejects the raw Tile output.
    nc = bacc.Bacc("TRN2", target_bir_lowering=False)

    x = nc.declare_dram_parameter("x", [C, HW], DT, isOutput=False)
    aux = nc.declare_dram_parameter("aux", [C, HW], DT, isOutput=False)
    Wq_w = nc.declare_dram_parameter("Wq_w", [CH, C], DT, isOutput=False)
    Wq_b = nc.declare_dram_parameter("Wq_b", [CH], DT, isOutput=False)
    Wk_w = nc.declare_dram_parameter("Wk_w", [CH, C], DT, isOutput=False)
    Wk_b = nc.declare_dram_parameter("Wk_b", [CH], DT, isOutput=False)
    Wv_w = nc.declare_dram_parameter("Wv_w", [CH, C], DT, isOutput=False)
    Wv_b = nc.declare_dram_parameter("Wv_b", [CH], DT, isOutput=False)
    Wz_w = nc.declare_dram_parameter("Wz_w", [C, CH], DT, isOutput=False)
    Wz_b = nc.declare_dram_parameter("Wz_b", [C], DT, isOutput=False)
    z = nc.declare_dram_parameter("z", [C, HW], DT, isOutput=True)

    groups = _groups()

    with tile.TileContext(nc) as tc:
        with (
            tc.tile_pool(name="consts", bufs=1) as consts,
            tc.tile_pool(name="sing", bufs=1) as sing,
            tc.tile_pool(name="expp", bufs=EXP_BUFS) as expp,
            tc.tile_pool(name="instream", bufs=6) as instream,
            tc.tile_pool(name="wload", bufs=2) as wload,
            tc.tile_pool(name="smalls", bufs=VTS_BUFS) as smalls,
            tc.tile_pool(name="xres", bufs=6) as xres,
            tc.tile_pool(name="zst", bufs=3) as zst,
            tc.tile_pool(name="psK", bufs=2, space="PSUM") as psK,
            tc.tile_pool(name="psY", bufs=2, space="PSUM") as psY,
        ):
            # ---- constants: biases, identity, transposed weights ----
            ident = consts.tile([P, P], DT)
            make_identity(nc, ident)
            ones_row = consts.tile([1, P], DT)
            nc.vector.memset(ones_row, 1.0)

            qb = consts.tile([P, 1], DT)
            nc.sync.dma_start(out=qb, in_=Wq_b[:].rearrange("(p o) -> p o", o=1))
            kb = consts.tile([P, 1], DT)
            nc.sync.dma_start(out=kb, in_=Wk_b[:].rearrange("(p o) -> p o", o=1))
            vb_row = consts.tile([1, P], DT)
            nc.sync.dma_start(out=vb_row, in_=Wv_b[:].rearrange("(o p) -> o p", o=1))
            zb0 = consts.tile([P, 1], DT)
            nc.sync.dma_start(out=zb0, in_=Wz_b[0:P].rearrange("(p o) -> p o", o=1))
            zb1 = consts.tile([P, 1], DT)
            nc.sync.dma_start(out=zb1, in_=Wz_b[P:C].rearrange("(p o) -> p o", o=1))
            zbias = (zb0, zb1)

            # transpose the three [CH, C] conv weights into [C, CH] halves,
            # and the [C, CH] projection into [CH, C] halves, via PE+identity
            WqT = consts.tile([P, 2, P], DT)
            WkT = consts.tile([P, 2, P], DT)
            WvT = consts.tile([P, 2, P], DT)
            WzT = consts.tile([P, 2, P], DT)
            for w_dram, w_t in ((Wq_w, WqT), (Wk_w, WkT), (Wv_w, WvT)):
                wt = wload.tile([P, C], DT, tag="wl")
                nc.sync.dma_start(out=wt, in_=w_dram[:, :])
                for h in range(2):
                    tp = psK.tile([P, P], DT, tag="kt")
                    nc.tensor.transpose(tp, wt[:, h * P : (h + 1) * P], ident)
                    nc.vector.tensor_copy(w_t[:, h], tp)
            for h in range(2):
                wt = wload.tile([P, P], DT, tag="wl")
                nc.sync.dma_start(out=wt, in_=Wz_w[h * P : (h + 1) * P, :])
                tp = psK.tile([P, P], DT, tag="kt")
                nc.tensor.transpose(tp, wt, ident)
                nc.vector.tensor_copy(WzT[:, h], tp)

            # broadcast Wv_b across partitions: bias_bcast[p, c] = Wv_b[c]
            bb_ps = psK.tile([P, P], DT, tag="kt")
            nc.tensor.matmul(bb_ps, ones_row, vb_row, start=True, stop=True)
            bias_bcast = consts.tile([P, P], DT)
            nc.vector.tensor_copy(bias_bcast, bb_ps)

            # ---- persistent [128, 4096] operands ----
            q_sb = sing.tile([P, HW], DT)
            k_sb = sing.tile([P, HW], DT)
            vT_sb = sing.tile([P, HW], DT)   # 32 tiles of [i=128, c=128]
            y_sb = sing.tile([P, HW], DT)

            # ---- phase 1: stream x/aux chunks, compute q, k, vT ----
            for cb in range(NJB):
                js = cb * 512
                a0 = instream.tile([P, 512], DT, tag="ins")
                nc.sync.dma_start(out=a0, in_=aux[0:P, js : js + 512])
                a1 = instream.tile([P, 512], DT, tag="ins")
                nc.sync.dma_start(out=a1, in_=aux[P:C, js : js + 512])
                x0 = instream.tile([P, 512], DT, tag="ins")
                nc.sync.dma_start(out=x0, in_=x[0:P, js : js + 512])
                x1 = instream.tile([P, 512], DT, tag="ins")
                nc.sync.dma_start(out=x1, in_=x[P:C, js : js + 512])

                kp = psK.tile([P, 512], DT, tag="kt")
                nc.tensor.matmul(kp, WkT[:, 0], a0, start=True, stop=False)
                nc.tensor.matmul(kp, WkT[:, 1], a1, start=False, stop=True)
                nc.vector.tensor_scalar_add(k_sb[:, js : js + 512], kp, kb)

                qp = psK.tile([P, 512], DT, tag="kt")
                nc.tensor.matmul(qp, WqT[:, 0], x0, start=True, stop=False)
                nc.tensor.matmul(qp, WqT[:, 1], x1, start=False, stop=True)
                nc.vector.tensor_scalar_add(q_sb[:, js : js + 512], qp, qb)

                for t in range(4):
                    it = cb * 4 + t
                    vp = psK.tile([P, P], DT, tag="kt")
                    nc.tensor.matmul(
                        vp, a0[:, t * P : (t + 1) * P], WvT[:, 0],
                        start=True, stop=False,
                    )
                    nc.tensor.matmul(
                        vp, a1[:, t * P : (t + 1) * P], WvT[:, 1],
                        start=False, stop=True,
                    )
                    nc.vector.tensor_add(
                        vT_sb[:, it * P : (it + 1) * P], vp, bias_bcast
                    )

            # ---- phase 2: attention ----
            exp_t: dict[int, bass.AP] = {}
            vts_t: dict[int, bass.AP] = {}

            def emit_a(it: int) -> None:
                """ktq + exp + row-sum + scaled-vT for one i-tile."""
                et = expp.tile([P, HW], DT, tag="exp")
                exp_t[it] = et
                sm = smalls.tile([P, len(EXP_CHUNKS)], DT, tag="sums")
                for ci, (off, w) in enumerate(EXP_CHUNKS):
                    kt = psK.tile([P, w], DT, tag="kt")
                    for s in range(w // 512):
                        nc.tensor.matmul(
                            kt[:, s * 512 : (s + 1) * 512],
                            k_sb[:, it * P : (it + 1) * P],
                            q_sb[:, off + s * 512 : off + (s + 1) * 512],
                            start=True, stop=True,
                        )
                    nc.scalar.activation(
                        out=et[:, off : off + w], in_=kt, func=Exp,
                        accum_out=sm[:, ci : ci + 1],
                    )
                sv = smalls.tile([P, 1], DT, tag="sv")
                nc.vector.reduce_sum(sv, sm, axis=AX)
                rv = smalls.tile([P, 1], DT, tag="rv")
                nc.vector.reciprocal(rv, sv)
                vt = smalls.tile([P, P], DT, tag="vts")
                nc.vector.tensor_scalar_mul(vt, vT_sb[:, it * P : (it + 1) * P], rv)
                vts_t[it] = vt

            def emit_b(g: int, jb: int) -> None:
                """y[:, jb] += vts.T @ exp for all i-tiles of group g."""
                js = jb * 512
                yp = psY.tile([P, 512], DT, tag="y")
                grp = groups[g]
                for gi, it in enumerate(grp):
                    nc.tensor.matmul(
                        yp, vts_t[it], exp_t[it][:, js : js + 512],
                        start=(gi == 0), stop=(gi == len(grp) - 1),
                    )
                if g == 0:
                    nc.vector.tensor_copy(y_sb[:, js : js + 512], yp)
                else:
                    nc.vector.tensor_add(y_sb[:, js : js + 512], y_sb[:, js : js + 512], yp)

            for g in range(len(groups)):
                # split the previous group's 8 y-blocks across this group's
                # A-emissions so PE keeps producing ktq chunks for ScalarE
                n_a = len(groups[g])
                base, extra = divmod(NJB, n_a)
                jb_cursor = 0
                for ai, it in enumerate(groups[g]):
                    emit_a(it)
                    if g > 0:
                        for _ in range(base + (1 if ai < extra else 0)):
                            emit_b(g - 1, jb_cursor)
                            jb_cursor += 1

            # ---- phase 3: last group's y + z = Wz @ y + zb + x ----
            xr: dict[tuple[int, int], bass.AP] = {}
            for jb in range(NJB):
                for h in range(2):
                    xt = xres.tile([P, 512], DT, tag="xr")
                    nc.sync.dma_start(
                        out=xt, in_=x[h * P : (h + 1) * P, jb * 512 : (jb + 1) * 512]
                    )
                    xr[(jb, h)] = xt

            for jb in range(NJB):
                emit_b(len(groups) - 1, jb)
                js = jb * 512
                for h in range(2):
                    zp = psY.tile([P, 512], DT, tag="y")
                    nc.tensor.matmul(
                        zp, WzT[:, h], y_sb[:, js : js + 512], start=True, stop=True
                    )
                    zc = zst.tile([P, 512], DT, tag="zc")
                    nc.scalar.activation(
                        out=zc, in_=zp, func=Identity, bias=zbias[h], scale=1.0
                    )
                    nc.vector.tensor_add(zc, zc, xr[(jb, h)])
                    nc.sync.dma_start(
                        out=z[h * P : (h + 1) * P, js : js + 512], in_=zc
                    )

    nc.compile()
    return nc


_NC = None


def _get_nc() -> bass.Bass:
    global _NC
    if _NC is None:
        _NC = build_module()
    return _NC


def _make_in_maps(inputs: dict[str, np.ndarray]) -> list[dict[str, np.ndarray]]:
    B = inputs["x"].shape[0]
    shared = {
        name: np.ascontiguousarray(np.asarray(inputs[name], dtype=np.float32))
        for name in ("Wq_w", "Wq_b", "Wk_w", "Wk_b", "Wv_w", "Wv_b", "Wz_w", "Wz_b")
    }
    in_maps = []
    for b in range(B):
        m = dict(shared)
        m["x"] = np.ascontiguousarray(
            np.asarray(inputs["x"][b], dtype=np.float32).reshape(C, HW)
        )
        m["aux"] = np.ascontiguousarray(
            np.asarray(inputs["aux"][b], dtype=np.float32).reshape(C, HW)
        )
        in_maps.append(m)
    return in_maps


def _install_ntff_hook_shim() -> None:
    """The agent image's antenv lacks axon_hooks; recreate it so
    run_bass_kernel_spmd(trace=True) can reach the libaxon NTFF profiler."""
    import types

    if "antenv.axon_hooks" in sys.modules:
        return
    import antenv

    mod = types.ModuleType("antenv.axon_hooks")
    state = {"hook": None}
    mod.set_axon_ntff_profile_hook = lambda h: state.__setitem__("hook", h)
    mod.get_axon_ntff_profile_hook = lambda: state["hook"]
    sys.modules["antenv.axon_hooks"] = mod
    antenv.axon_hooks = mod
    try:
        from trn_agent_boot.trn_boot import _ntff_profile_via_ctypes

        hook = _ntff_profile_via_ctypes("/opt/axon/libaxon_pjrt.so")
        if hook is not None:
            mod.set_axon_ntff_profile_hook(hook)
    except Exception as e:  # degrade to no tracing
        print(f"ntff hook unavailable: {e}", file=sys.stderr)


def run(inputs: dict[str, np.ndarray], trace: bool = False):
    """Run on the 8 NeuronCores; returns (output [8,256,64,64], BassKernelResults)."""
    from concourse.bass_utils import run_bass_kernel_spmd

    if trace:
        _install_ntff_hook_shim()
    nc = _get_nc()
    in_maps = _make_in_maps(inputs)
    res = run_bass_kernel_spmd(nc, in_maps, list(range(len(in_maps))), trace=trace)
    out = np.stack([r["z"].reshape(C, 64, 64) for r in res.results])
    return out.astype(np.float32), res


def kernel(**inputs: np.ndarray) -> np.ndarray:
    out, _ = run(inputs, trace=False)
    return out


if __name__ == "__main__":
    nc = build_module()
    print("module built ok")


# revision 7
# speedup vs baseline: 2.0650x; 2.0650x over previous
"""Trainium2 Bass kernel for nn_AttentionV2 (dense transformer attention block).

Reference computation (per batch element b):
    q  = Wq @ x_b  + qb          # [128, 4096]  (1x1 conv over channels)
    k  = Wk @ aux_b + kb         # [128, 4096]
    v  = Wv @ aux_b + vb         # [128, 4096]
    ktq[i, j] = sum_c k[c, i] * q[c, j]          # [4096, 4096]
    atten = softmax(ktq, axis=j)
    y[c, j] = sum_i v[c, i] * atten[i, j]        # [128, 4096]
    z = Wz @ y + zb + x_b        # [256, 4096]

Sharding: batch B=8 across the 8 cores (data parallel, weights replicated).
Each core runs the whole attention for its batch element; no collectives.

Per-core design notes:
  * All matmuls contract over the partition dim; k/q land as [c=128, hw] so
    ktq tiles need no transposes.  v is produced directly transposed
    (vT[i, c]) by using aux as the stationary operand, so the attention
    matmul y = vT.T @ exp needs no transpose either.
  * Softmax is computed unnormalized (no max subtraction: |ktq| <~ 30 so
    exp stays finite in fp32); the 1/rowsum factor is folded into vT
    (per-partition scalar multiply), which makes normalization free.
  * ScalarE computes exp straight out of PSUM with accum_out producing the
    row sums, so the only DVE work is the y accumulation flushes.
  * PSUM budget (8 banks): 2 x [128, 1536] ktq chunks (6 banks) feed exp;
    2 x [128, 512] (2 banks) rotate for the grouped y accumulation and the
    final z conv.
  * i-tiles are processed in groups of G=3; the y matmuls of group g are
    interleaved (in emission order) with the ktq/exp work of group g+1 so
    the tensor engine keeps ScalarE fed at the group boundary.
"""

import sys

if "/opt/trn_rl_repo" not in sys.path:
    sys.path.insert(0, "/opt/trn_rl_repo")

import numpy as np

import concourse.bass as bass
import concourse.bacc as bacc
import concourse.mybir as mybir
import concourse.tile as tile
from concourse.masks import make_identity

DT = mybir.dt.float32
P = 128          # partitions
C = 256          # input channels
CH = 128         # conv output channels (C//2)
HW = 4096        # 64*64 spatial
NJB = HW // 512  # 8 column blocks of 512
NIT = HW // P    # 32 i-tiles
G = 3            # i-tiles per y-accumulation group
# exp is computed in chunks straight out of PSUM; chunk layout per i-tile:
EXP_CHUNKS = ((0, 1536), (1536, 1536), (3072, 1024))
KT_SLOT = 1536   # psum slot width for ktq chunks (3 banks)

EXP_BUFS = 5
VTS_BUFS = 6

R32 = mybir.dt.float32r


def _r(ap):
    """View an fp32 AP as float32r: single-pass PE matmul (1 cyc/row at
    N>=256) instead of fp32's 4-pass. Storage unchanged."""
    return ap.bitcast(R32)


Exp = mybir.ActivationFunctionType.Exp
Identity = mybir.ActivationFunctionType.Identity
AX = mybir.AxisListType.X


def _groups():
    return [list(range(s, min(s + G, NIT))) for s in range(0, NIT, G)]


def build_module() -> bass.Bass:
    # Bacc (not plain Bass): its compile() pipeline moves extra matmul waits
    # onto LDWEIGHTS and splits >1-wait instructions (TRN2 ISA allows one
    # sync wait per instruction) -- walrus rejects the raw Tile output.
    nc = bacc.Bacc("TRN2", target_bir_lowering=False)

    x = nc.declare_dram_parameter("x", [C, HW], DT, isOutput=False)
    aux = nc.declare_dram_parameter("aux", [C, HW], DT, isOutput=False)
    Wq_w = nc.declare_dram_parameter("Wq_w", [CH, C], DT, isOutput=False)
    Wq_b = nc.declare_dram_parameter("Wq_b", [CH], DT, isOutput=False)
    Wk_w = nc.declare_dram_parameter("Wk_w", [CH, C], DT, isOutput=False)
    Wk_b = nc.declare_dram_parameter("Wk_b", [CH], DT, isOutput=False)
    Wv_w = nc.declare_dram_parameter("Wv_w", [CH, C], DT, isOutput=False)
    Wv_b = nc.declare_dram_parameter("Wv_b", [CH], DT, isOutput=False)
    Wz_w = nc.declare_dram_parameter("Wz_w", [C, CH], DT, isOutput=False)
    Wz_b = nc.declare_dram_parameter("Wz_b", [C], DT, isOutput=False)
    z = nc.declare_dram_parameter("z", [C, HW], DT, isOutput=True)

    groups = _groups()

    with tile.TileContext(nc) as tc:
        with (
            tc.tile_pool(name="consts", bufs=1) as consts,
            tc.tile_pool(name="sing", bufs=1) as sing,
            tc.tile_pool(name="expp", bufs=EXP_BUFS) as expp,
            tc.tile_pool(name="instream", bufs=6) as instream,
            tc.tile_pool(name="wload", bufs=2) as wload,
            tc.tile_pool(name="smalls", bufs=VTS_BUFS) as smalls,
            tc.tile_pool(name="xres", bufs=6) as xres,
            tc.tile_pool(name="zst", bufs=3) as zst,
            tc.tile_pool(name="psK", bufs=2, space="PSUM") as psK,
            tc.tile_pool(name="psY", bufs=2, space="PSUM") as psY,
        ):
            # ---- constants: biases, identity, transposed weights ----
            ident = consts.tile([P, P], DT)
            make_identity(nc, ident)
            ones_row = consts.tile([1, P], DT)
            nc.vector.memset(ones_row, 1.0)

            qb = consts.tile([P, 1], DT)
            nc.sync.dma_start(out=qb, in_=Wq_b[:].rearrange("(p o) -> p o", o=1))
            kb = consts.tile([P, 1], DT)
            nc.sync.dma_start(out=kb, in_=Wk_b[:].rearrange("(p o) -> p o", o=1))
            vb_row = consts.tile([1, P], DT)
            nc.sync.dma_start(out=vb_row, in_=Wv_b[:].rearrange("(o p) -> o p", o=1))
            zb0 = consts.tile([P, 1], DT)
            nc.sync.dma_start(out=zb0, in_=Wz_b[0:P].rearrange("(p o) -> p o", o=1))
            zb1 = consts.tile([P, 1], DT)
            nc.sync.dma_start(out=zb1, in_=Wz_b[P:C].rearrange("(p o) -> p o", o=1))
            zbias = (zb0, zb1)

            # transpose the three [CH, C] conv weights into [C, CH] halves,
            # and the [C, CH] projection into [CH, C] halves, via PE+identity
            WqT = consts.tile([P, 2, P], DT)
            WkT = consts.tile([P, 2, P], DT)
            WvT = consts.tile([P, 2, P], DT)
            WzT = consts.tile([P, 2, P], R32)
            for w_dram, w_t in ((Wq_w, WqT), (Wk_w, WkT), (Wv_w, WvT)):
                wt = wload.tile([P, C], DT, tag="wl")
                nc.sync.dma_start(out=wt, in_=w_dram[:, :])
                for h in range(2):
                    tp = psK.tile([P, P], DT, tag="kt")
                    nc.tensor.transpose(tp, wt[:, h * P : (h + 1) * P], ident)
                    nc.vector.tensor_copy(w_t[:, h], tp)
            for h in range(2):
                wt = wload.tile([P, P], DT, tag="wl")
                nc.sync.dma_start(out=wt, in_=Wz_w[h * P : (h + 1) * P, :])
                tp = psK.tile([P, P], DT, tag="kt")
                nc.tensor.transpose(tp, wt, ident)
                nc.vector.tensor_copy(WzT[:, h], tp)

            # broadcast Wv_b across partitions: bias_bcast[p, c] = Wv_b[c]
            bb_ps = psK.tile([P, P], DT, tag="kt")
            nc.tensor.matmul(bb_ps, ones_row, vb_row, start=True, stop=True)
            bias_bcast = consts.tile([P, P], DT)
            nc.vector.tensor_copy(bias_bcast, bb_ps)

            # ---- persistent [128, 4096] operands ----
            # q/k/y and the exp tiles are written by DVE/ACT with float32r out
            # dtype (rounds mantissa) so the PE can consume them single-pass.
            q_sb = sing.tile([P, HW], R32)
            k_sb = sing.tile([P, HW], R32)
            vT_sb = sing.tile([P, HW], DT)   # 32 tiles of [i=128, c=128]
            y_sb = sing.tile([P, HW], R32)

            # ---- phase 1: stream x/aux chunks, compute q, k, vT ----
            for cb in range(NJB):
                js = cb * 512
                a0 = instream.tile([P, 512], DT, tag="ins")
                nc.sync.dma_start(out=a0, in_=aux[0:P, js : js + 512])
                a1 = instream.tile([P, 512], DT, tag="ins")
                nc.sync.dma_start(out=a1, in_=aux[P:C, js : js + 512])
                x0 = instream.tile([P, 512], DT, tag="ins")
                nc.sync.dma_start(out=x0, in_=x[0:P, js : js + 512])
                x1 = instream.tile([P, 512], DT, tag="ins")
                nc.sync.dma_start(out=x1, in_=x[P:C, js : js + 512])

                kp = psK.tile([P, 512], DT, tag="kt")
                nc.tensor.matmul(kp, WkT[:, 0], a0, start=True, stop=False)
                nc.tensor.matmul(kp, WkT[:, 1], a1, start=False, stop=True)
                nc.vector.tensor_scalar_add(k_sb[:, js : js + 512], kp, kb)

                qp = psK.tile([P, 512], DT, tag="kt")
                nc.tensor.matmul(qp, WqT[:, 0], x0, start=True, stop=False)
                nc.tensor.matmul(qp, WqT[:, 1], x1, start=False, stop=True)
                nc.vector.tensor_scalar_add(q_sb[:, js : js + 512], qp, qb)

                for t in range(4):
                    it = cb * 4 + t
                    vp = psK.tile([P, P], DT, tag="kt")
                    nc.tensor.matmul(
                        vp, a0[:, t * P : (t + 1) * P], WvT[:, 0],
                        start=True, stop=False,
                    )
                    nc.tensor.matmul(
                        vp, a1[:, t * P : (t + 1) * P], WvT[:, 1],
                        start=False, stop=True,
                    )
                    nc.vector.tensor_add(
                        vT_sb[:, it * P : (it + 1) * P], vp, bias_bcast
                    )

            # ---- phase 2: attention ----
            exp_t: dict[int, bass.AP] = {}
            vts_t: dict[int, bass.AP] = {}

            def emit_a(it: int) -> None:
                """ktq + exp + row-sum + scaled-vT for one i-tile."""
                et = expp.tile([P, HW], R32, tag="exp")
                exp_t[it] = et
                sm = smalls.tile([P, len(EXP_CHUNKS)], DT, tag="sums")
                for ci, (off, w) in enumerate(EXP_CHUNKS):
                    kt = psK.tile([P, w], DT, tag="kt")
                    for s in range(w // 512):
                        nc.tensor.matmul(
                            kt[:, s * 512 : (s + 1) * 512],
                            _r(k_sb[:, it * P : (it + 1) * P]),
                            _r(q_sb[:, off + s * 512 : off + (s + 1) * 512]),
                            start=True, stop=True,
                        )
                    nc.scalar.activation(
                        out=et[:, off : off + w], in_=kt, func=Exp,
                        accum_out=sm[:, ci : ci + 1],
                    )
                sv = smalls.tile([P, 1], DT, tag="sv")
                nc.vector.reduce_sum(sv, sm, axis=AX)
                rv = smalls.tile([P, 1], DT, tag="rv")
                nc.vector.reciprocal(rv, sv)
                vt = smalls.tile([P, P], R32, tag="vts")
                nc.vector.tensor_scalar_mul(vt, vT_sb[:, it * P : (it + 1) * P], rv)
                vts_t[it] = vt

            def emit_b(g: int, jb: int) -> None:
                """y[:, jb] += vts.T @ exp for all i-tiles of group g."""
                js = jb * 512
                yp = psY.tile([P, 512], DT, tag="y")
                grp = groups[g]
                for gi, it in enumerate(grp):
                    nc.tensor.matmul(
                        yp, _r(vts_t[it]), _r(exp_t[it][:, js : js + 512]),
                        start=(gi == 0), stop=(gi == len(grp) - 1),
                    )
                if g == 0:
                    nc.vector.tensor_copy(y_sb[:, js : js + 512], yp)
                else:
                    nc.vector.tensor_add(y_sb[:, js : js + 512], y_sb[:, js : js + 512], yp)

            for g in range(len(groups)):
                # split the previous group's 8 y-blocks across this group's
                # A-emissions so PE keeps producing ktq chunks for ScalarE
                n_a = len(groups[g])
                base, extra = divmod(NJB, n_a)
                jb_cursor = 0
                for ai, it in enumerate(groups[g]):
                    emit_a(it)
                    if g > 0:
                        for _ in range(base + (1 if ai < extra else 0)):
                            emit_b(g - 1, jb_cursor)
                            jb_cursor += 1

            # ---- phase 3: last group's y + z = Wz @ y + zb + x ----
            xr: dict[tuple[int, int], bass.AP] = {}
            for jb in range(NJB):
                for h in range(2):
                    xt = xres.tile([P, 512], DT, tag="xr")
                    nc.sync.dma_start(
                        out=xt, in_=x[h * P : (h + 1) * P, jb * 512 : (jb + 1) * 512]
                    )
                    xr[(jb, h)] = xt

            for jb in range(NJB):
                emit_b(len(groups) - 1, jb)
                js = jb * 512
                for h in range(2):
                    zp = psY.tile([P, 512], DT, tag="y")
                    nc.tensor.matmul(
                        zp, _r(WzT[:, h]), _r(y_sb[:, js : js + 512]), start=True, stop=True
                    )
                    zc = zst.tile([P, 512], DT, tag="zc")
                    nc.scalar.activation(
                        out=zc, in_=zp, func=Identity, bias=zbias[h], scale=1.0
                    )
                    nc.vector.tensor_add(zc, zc, xr[(jb, h)])
                    nc.sync.dma_start(
                        out=z[h * P : (h + 1) * P, js : js + 512], in_=zc
                    )

    nc.compile()
    return nc


_NC = None


def _get_nc() -> bass.Bass:
    global _NC
    if _NC is None:
        _NC = build_module()
    return _NC


def _make_in_maps(inputs: dict[str, np.ndarray]) -> list[dict[str, np.ndarray]]:
    B = inputs["x"].shape[0]
    shared = {
        name: np.ascontiguousarray(np.asarray(inputs[name], dtype=np.float32))
        for name in ("Wq_w", "Wq_b", "Wk_w", "Wk_b", "Wv_w", "Wv_b", "Wz_w", "Wz_b")
    }
    in_maps = []
    for b in range(B):
        m = dict(shared)
        m["x"] = np.ascontiguousarray(
            np.asarray(inputs["x"][b], dtype=np.float32).reshape(C, HW)
        )
        m["aux"] = np.ascontiguousarray(
            np.asarray(inputs["aux"][b], dtype=np.float32).reshape(C, HW)
        )
        in_maps.append(m)
    return in_maps


def _install_ntff_hook_shim() -> None:
    """The agent image's antenv lacks axon_hooks; recreate it so
    run_bass_kernel_spmd(trace=True) can reach the libaxon NTFF profiler."""
    import types

    if "antenv.axon_hooks" in sys.modules:
        return
    import antenv

    mod = types.ModuleType("antenv.axon_hooks")
    state = {"hook": None}
    mod.set_axon_ntff_profile_hook = lambda h: state.__setitem__("hook", h)
    mod.get_axon_ntff_profile_hook = lambda: state["hook"]
    sys.modules["antenv.axon_hooks"] = mod
    antenv.axon_hooks = mod
    try:
        from trn_agent_boot.trn_boot import _ntff_profile_via_ctypes

        hook = _ntff_profile_via_ctypes("/opt/axon/libaxon_pjrt.so")
        if hook is not None:
            mod.set_axon_ntff_profile_hook(hook)
    except Exception as e:  # degrade to no tracing
        print(f"ntff hook unavailable: {e}", file=sys.stderr)


def run(inputs: dict[str, np.ndarray], trace: bool = False):
    """Run on the 8 NeuronCores; returns (output [8,256,64,64], BassKernelResults)."""
    from concourse.bass_utils import run_bass_kernel_spmd

    if trace:
        _install_ntff_hook_shim()
    nc = _get_nc()
    in_maps = _make_in_maps(inputs)
    res = run_bass_kernel_spmd(nc, in_maps, list(range(len(in_maps))), trace=trace)
    out = np.stack([r["z"].reshape(C, 64, 64) for r in res.results])
    return out.astype(np.float32), res


def kernel(**inputs: np.ndarray) -> np.ndarray:
    out, _ = run(inputs, trace=False)
    return out


if __name__ == "__main__":
    nc = build_module()
    print("module built ok")
